# revision 16
# baseline (speedup 1.0000x reference)
"""Causal self-attention (QK-RMSNorm + RoPE) on 8 Trainium2 NeuronCores.

Problem: x[2,2048,2048], Wq/Wk/Wv/Wo [2048,2048], 16 heads, head_dim 128.

Sharding: core c handles batch b=c//4 and head group g=c%4 (4 heads,
model cols [512g:512g+512)).  QKV projections are computed from the
host-pre-transposed xT (contraction dim on partitions).  Attention uses
transposed scores (eT = exp(scale * kT_blk.T @ qT_chunk)), so the AV
matmul (lhsT=v, rhs=eT) directly yields the transposed attention output
yT[d, i] that o_proj consumes.  The softmax denominator comes from a
ones-lhsT matmul over eT, its reciprocal is broadcast across partitions
with a K=1 PE matmul.  Per-batch groups of 4 cores AllGather their
yT head shards, then each core computes a 512-column slice of the
output projection (y = attn @ Wo.T) in transposed layout.  The host
de-transposes and concatenates.  No AllReduce is needed.

Matmuls run in fp32r (full PE rate, ~1.5e-4 rel err).
"""

import math
from contextlib import ExitStack

import numpy as np

import concourse.bass as bass
import concourse.bacc as bacc
import concourse.tile as tile
from concourse import mybir
from concourse.bass_utils import run_bass_kernel_spmd
from concourse.masks import make_identity

P = 128
D = 2048
S = 2048
HD = 128              # head dim
NHL = 4               # heads per core
GW = NHL * HD         # 512, per-core width of head group
CT = D // P           # 16 contraction tiles
ICH = 4               # i-chunks of 512 positions
NCORES = 8
F32 = mybir.dt.float32
F16 = mybir.dt.float16
F32R = mybir.dt.float32r
SCALE = 1.0 / math.sqrt(HD)
EPS = 1.1920928955078125e-07
MASK_NEG = -30000.0

_program_cache = {}


def build_program():
    if "nc" in _program_cache:
        return _program_cache["nc"]

    nc = bacc.Bacc("TRN2", target_bir_lowering=False, debug=False, num_devices=NCORES)

    xt_in = nc.dram_tensor("xt", [D, S], F16, kind="ExternalInput")
    wq_in = nc.dram_tensor("wq", [D, GW], F16, kind="ExternalInput")
    wk_in = nc.dram_tensor("wk", [D, GW], F16, kind="ExternalInput")
    wv_in = nc.dram_tensor("wv", [D, GW], F16, kind="ExternalInput")
    wo_in = nc.dram_tensor("wo", [D, GW], F16, kind="ExternalInput")
    cos_in = nc.dram_tensor("cos", [S, HD // 2], F16, kind="ExternalInput")
    sin_in = nc.dram_tensor("sin", [S, HD // 2], F16, kind="ExternalInput")
    mask_in = nc.dram_tensor("maskt", [4, P, 512], F16, kind="ExternalInput")
    roff_in = nc.dram_tensor("roff", [1, 2], mybir.dt.uint32, kind="ExternalInput")
    yt_out = nc.dram_tensor("yt_out", [GW, S], F32, kind="ExternalOutput")

    with tile.TileContext(nc) as tc:
        with ExitStack() as ctx:
            const = ctx.enter_context(tc.tile_pool(name="const", bufs=1))
            dram = ctx.enter_context(tc.tile_pool(name="dram", bufs=1, space="DRAM"))

            ident = const.tile([P, P], F16, name="ident")
            make_identity(nc, ident)
            eps_t = const.tile([P, 1], F32, name="eps_t")
            nc.vector.memset(eps_t[:], EPS)
            zero_t = const.tile([P, 1], F32, name="zero_t")
            nc.vector.memset(zero_t[:], 0.0)
            neg1_t = const.tile([P, 1], F32, name="neg1_t")
            nc.vector.memset(neg1_t[:], -1.0)
            ones_f = const.tile([P, P], F32, name="ones_f")
            nc.vector.memset(ones_f[:], 1.0)
            ones2 = const.tile([P, 2], F16, name="ones2")
            nc.scalar.copy(ones2[:], ones_f[:, 0:2])
            ones_row = const.tile([1, P], F32R, name="ones_row")
            nc.scalar.copy(ones_row[:], ones_f[0:1, :])

            cos_sb = const.tile([P, CT, HD // 2], F16, name="cos_sb")
            nc.sync.dma_start(out=cos_sb[:], in_=cos_in.ap().rearrange("(a p) f -> p a f", p=P))
            sin_sb = const.tile([P, CT, HD // 2], F16, name="sin_sb")
            nc.sync.dma_start(out=sin_sb[:], in_=sin_in.ap().rearrange("(a p) f -> p a f", p=P))

            qt_d = dram.tile([GW, S], F16, name="qt_d")
            kt_d = dram.tile([GW, S], F16, name="kt_d")
            yt_ics = [dram.tile([GW, 512], F16, name=f"yt_ic{i}") for i in range(ICH)]
            ag_ics = [
                dram.tile([NCORES * GW, 512], F16, name=f"ag_ic{i}", addr_space="Shared")
                for i in range(ICH)
            ]

            # wo pool created up-front so phase A can prefetch into it
            wopool = ctx.enter_context(tc.tile_pool(name="wopool", bufs=1))
            wo_sb = wopool.tile([P, CT, GW], F16, name="wo_sb")

            # ---------------- Phase A: Q and K ----------------
            with ExitStack() as pha:
                wpool = pha.enter_context(tc.tile_pool(name="wpool", bufs=1))
                xt_pool = pha.enter_context(tc.tile_pool(name="xt_pool", bufs=2))
                proj_ps = pha.enter_context(tc.tile_pool(name="proj_ps", bufs=2, space="PSUM"))
                tp_ps = pha.enter_context(tc.tile_pool(name="tp_ps", bufs=2, space="PSUM"))
                rope = pha.enter_context(tc.tile_pool(name="rope", bufs=2))
                stat = pha.enter_context(tc.tile_pool(name="stat", bufs=2))
                evq = pha.enter_context(tc.tile_pool(name="evq", bufs=3))

                wq_sb = wpool.tile([P, CT, GW], F16, name="wq_sb")
                wk_sb = wpool.tile([P, CT, GW], F16, name="wk_sb")
                # prefetch O weights late in phase A (pool lives in outer scope)
                for ica in range(8):
                    xt_ch = xt_pool.tile([P, CT, 256], F16, name=f"xt_ch{ica}", tag="xt")
                    for ct in range(CT):
                        if ica == 0:
                            # interleave weight-tile and first-chunk loads so the
                            # first accumulation groups unblock ct-by-ct
                            nc.sync.dma_start(out=wq_sb[:, ct, :], in_=wq_in[ct * P:(ct + 1) * P, :])
                        nc.sync.dma_start(
                            out=xt_ch[:, ct, :],
                            in_=xt_in[ct * P:(ct + 1) * P, ica * 256:(ica + 1) * 256],
                        )
                        if ica == 0:
                            nc.sync.dma_start(out=wk_sb[:, ct, :], in_=wk_in[ct * P:(ct + 1) * P, :])
                        if ica == 1:
                            nc.sync.dma_start(out=wo_sb[:, ct, :], in_=wo_in[ct * P:(ct + 1) * P, :])
                    for ib in range(2):
                        ibg = ica * 2 + ib         # global i-block
                        i0 = ibg * P
                        for wsb, dst in ((wq_sb, qt_d), (wk_sb, kt_d)):
                            ps = proj_ps.tile([P, GW], F32, name=f"ps{ibg}", tag="proj")
                            for ct in range(CT):
                                nc.tensor.matmul(
                                    ps[:],
                                    xt_ch[:, ct, ib * P:(ib + 1) * P],
                                    wsb[:, ct, :],
                                    start=(ct == 0),
                                    stop=(ct == CT - 1),
                                )
                            # rms norm stats + rope, all row-wise
                            qs = rope.tile([P, GW], F16, name=f"qs{ibg}", tag="qs")
                            nc.scalar.copy(qs[:], ps[:])
                            sq = rope.tile([P, GW], F16, name=f"sq{ibg}", tag="sq")
                            nc.vector.tensor_mul(sq[:], qs[:], qs[:])
                            rstd = stat.tile([P, NHL], F32, name=f"rstd{ibg}", tag="rstd")
                            for h in range(NHL):
                                nc.vector.reduce_sum(
                                    rstd[:, h:h + 1],
                                    sq[:, h * HD:(h + 1) * HD],
                                    axis=mybir.AxisListType.X,
                                )
                            nc.scalar.activation(
                                rstd[:], rstd[:],
                                mybir.ActivationFunctionType.Sqrt,
                                bias=eps_t[:], scale=1.0 / HD,
                            )
                            nc.vector.reciprocal(rstd[:], rstd[:])

                            q3 = qs[:].rearrange("p (h d) -> p h d", h=NHL)
                            qr = rope.tile([P, GW], F16, name=f"qr{ibg}", tag="qr")
                            qr3 = qr[:].rearrange("p (h d) -> p h d", h=NHL)
                            tmp = rope.tile([P, NHL, HD // 2], F16, name=f"tmp{ibg}", tag="tmp")
                            cosB = cos_sb[:, ibg:ibg + 1, :].broadcast_to((P, NHL, HD // 2))
                            sinB = sin_sb[:, ibg:ibg + 1, :].broadcast_to((P, NHL, HD // 2))
                            h1 = q3[:, :, 0:HD // 2]
                            h2 = q3[:, :, HD // 2:HD]
                            # r1 = q1*cos + q2*sin ; r2 = q2*cos - q1*sin
                            nc.vector.tensor_mul(qr3[:, :, 0:HD // 2], h1, cosB)
                            nc.vector.tensor_mul(tmp[:], h2, sinB)
                            nc.vector.tensor_add(qr3[:, :, 0:HD // 2], qr3[:, :, 0:HD // 2], tmp[:])
                            nc.vector.tensor_mul(qr3[:, :, HD // 2:HD], h2, cosB)
                            nc.vector.tensor_mul(tmp[:], h1, sinB)
                            nc.vector.tensor_sub(
                                qr3[:, :, HD // 2:HD], qr3[:, :, HD // 2:HD], tmp[:]
                            )
                            for h in range(NHL):
                                nc.vector.tensor_scalar_mul(
                                    qr[:, h * HD:(h + 1) * HD],
                                    qr[:, h * HD:(h + 1) * HD],
                                    rstd[:, h:h + 1],
                                )
                            # transpose per head, evict to DRAM
                            for h in range(NHL):
                                tp = tp_ps.tile([P, P], F16, name=f"tp{ibg}_{h}", tag="tp")
                                nc.tensor.transpose(tp[:], qr[:, h * HD:(h + 1) * HD], ident[:])
                                qtt = evq.tile([P, P], F16, name=f"qtt{ibg}_{h}", tag="qtt")
                                nc.scalar.copy(qtt[:], tp[:])
                                nc.scalar.dma_start(
                                    out=dst[h * P:(h + 1) * P, i0:i0 + P], in_=qtt[:]
                                )

            # wv/wo/v pools: wo persists to phase D; wv+v live A..B
            # (their DMAs are issued inside phase A for prefetch)

            # ---------------- Phase A2: V ----------------
            with ExitStack() as phab:
                # v + mask live A2..B; freed before phase D
                vpool = phab.enter_context(tc.tile_pool(name="vpool", bufs=1))
                v_sb = vpool.tile([P, CT, GW], F16, name="v_sb")
                maskp = phab.enter_context(tc.tile_pool(name="maskp", bufs=1))
                mask_sb = maskp.tile([P, 4, 512], F16, name="mask_sb")
                nc.sync.dma_start(out=mask_sb[:], in_=mask_in.ap().rearrange("t p f -> p t f"))

                wvpool = phab.enter_context(tc.tile_pool(name="wvpool", bufs=1))
                xt2_pool = phab.enter_context(tc.tile_pool(name="xt2_pool", bufs=2))
                wv_sb = wvpool.tile([P, CT, GW], F16, name="wv_sb")
                for ct in range(CT):
                    nc.sync.dma_start(out=wv_sb[:, ct, :], in_=wv_in[ct * P:(ct + 1) * P, :])

                # ---------------- Phase B: attention, i-chunk outer ----------------
                with ExitStack() as phb:
                    kt_pool = phb.enter_context(tc.tile_pool(name="kt_pool", bufs=2))
                    qt_pool = phb.enter_context(tc.tile_pool(name="qt_pool", bufs=2))
                    et_pool = phb.enter_context(tc.tile_pool(name="et_pool", bufs=4))
                    s_ps = phb.enter_context(tc.tile_pool(name="s_ps", bufs=2, space="PSUM"))
                    v_psp = phb.enter_context(tc.tile_pool(name="v_psp", bufs=1, space="PSUM"))
                    acc_ps = phb.enter_context(tc.tile_pool(name="acc_ps", bufs=2, space="PSUM"))
                    bc_psp = phb.enter_context(tc.tile_pool(name="bc_psp", bufs=1, space="PSUM"))
                    bsmall = phb.enter_context(tc.tile_pool(name="bsmall", bufs=2))

                    for ic in range(ICH):
                        njb = 4 * ic + 4
                        # just-in-time V projection for this chunk's 4 j-blocks
                        xt_ch = xt2_pool.tile([P, CT, 512], F16, name=f"x2_{ic}", tag="xt2")
                        for ct in range(CT):
                            nc.sync.dma_start(
                                out=xt_ch[:, ct, :],
                                in_=xt_in[ct * P:(ct + 1) * P, ic * 512:(ic + 1) * 512],
                            )
                        for ib in range(4):
                            jbv = ic * 4 + ib
                            vps = v_psp.tile([P, GW], F32, name=f"vps{jbv}", tag="vps")
                            for ct in range(CT):
                                nc.tensor.matmul(
                                    vps[:],
                                    xt_ch[:, ct, ib * P:(ib + 1) * P],
                                    wv_sb[:, ct, :],
                                    start=(ct == 0),
                                    stop=(ct == CT - 1),
                                )
                            nc.scalar.copy(v_sb[:, jbv, :], vps[:])
                        kt_t = kt_pool.tile([P, NHL, 2048], F16, name=f"kt{ic}", tag="kt")
                        qt_t = qt_pool.tile([P, NHL, 512], F16, name=f"qt{ic}", tag="qt")
                        for h in range(NHL):
                            nc.sync.dma_start(
                                out=kt_t[:, h, 0:njb * P],
                                in_=kt_d[h * P:(h + 1) * P, 0:njb * P],
                            )
                            nc.sync.dma_start(
                                out=qt_t[:, h, :],
                                in_=qt_d[h * P:(h + 1) * P, ic * 512:(ic + 1) * 512],
                            )
                        for h in range(NHL):
                            yt_ps = acc_ps.tile([P, 512], F32, name=f"yt{h}_{ic}", tag="yt")
                            den_ps = acc_ps.tile([2, 512], F32, name=f"den{h}_{ic}", tag="den")
                            for jb in range(njb):
                                sp = s_ps.tile([P, 512], F32, name=f"s{h}_{ic}_{jb}", tag="s")
                                nc.tensor.matmul(
                                    sp[:],
                                    kt_t[:, h, jb * P:(jb + 1) * P],
                                    qt_t[:, h, :],
                                    start=True, stop=True,
                                )
                                et = et_pool.tile([P, 512], F16, name=f"et{h}_{ic}_{jb}", tag="et")
                                nc.scalar.activation(
                                    et[:], sp[:],
                                    mybir.ActivationFunctionType.Exp,
                                    bias=neg1_t[:], scale=SCALE,
                                )
                                t = jb - 4 * ic
                                if t >= 0:
                                    nc.vector.tensor_mul(et[:], et[:], mask_sb[:, t, :])
                                nc.tensor.matmul(
                                    yt_ps[:],
                                    v_sb[:, jb, h * HD:(h + 1) * HD],
                                    et[:],
                                    start=(jb == 0), stop=(jb == njb - 1),
                                )
                                nc.tensor.matmul(
                                    den_ps[:],
                                    ones2[:],
                                    et[:],
                                    start=(jb == 0), stop=(jb == njb - 1),
                                )
                            rden = bsmall.tile([1, 512], F32, name=f"rd{h}_{ic}", tag="rden")
                            nc.vector.reciprocal(rden[:], den_ps[0:1, :])
                            rden_r = bsmall.tile([1, 512], F32R, name=f"rdr{h}_{ic}", tag="rdenr")
                            nc.scalar.copy(rden_r[:], rden[:])
                            bc_ps = bc_psp.tile([P, 512], F32, name=f"bc{h}_{ic}", tag="bc")
                            nc.tensor.matmul(bc_ps[:], ones_row[:], rden_r[:], start=True, stop=True)
                            bc_sb = bsmall.tile([P, 512], F32, name=f"bcs{h}_{ic}", tag="bcs")
                            nc.vector.tensor_copy(bc_sb[:], bc_ps[:])
                            yt_sb = bsmall.tile([P, 512], F16, name=f"yts{h}_{ic}", tag="yts")
                            nc.vector.tensor_mul(yt_sb[:], yt_ps[:], bc_sb[:])
                            nc.gpsimd.dma_start(
                                out=yt_ics[ic][h * P:(h + 1) * P, :],
                                in_=yt_sb[:],
                            )
                        # per-chunk AllGather fires as soon as chunk ic is written
                        nc.gpsimd.collective_compute(
                            "AllGather",
                            mybir.AluOpType.bypass,
                            replica_groups=[list(range(NCORES))],
                            ins=[yt_ics[ic][:].opt()],
                            outs=[ag_ics[ic][:].opt()],
                        )

            # ---------------- Phase D: o_proj (column shard), pipelined per chunk ----------------
            with ExitStack() as phd:
                ag_pool = phd.enter_context(tc.tile_pool(name="ag_pool", bufs=2))
                d_ps = phd.enter_context(tc.tile_pool(name="d_ps", bufs=2, space="PSUM"))
                ev2 = phd.enter_context(tc.tile_pool(name="ev2", bufs=3))
                roffp = phd.enter_context(tc.tile_pool(name="roffp", bufs=1))

                roff_sb = roffp.tile([1, 2], mybir.dt.uint32, name="roff_sb")
                nc.sync.dma_start(out=roff_sb[:], in_=roff_in[:, :])
                roff_reg = nc.alloc_registers()
                nc.regs_load(roff_reg, roff_sb[0:1, 0:1])
                rv = nc.snap(roff_reg, donate=True)

                for icc in range(ICH):
                    ag_ch = ag_pool.tile([P, CT, 512], F16, name=f"ag{icc}", tag="ag")
                    nc.sync.dma_start(
                        out=ag_ch[:],
                        in_=ag_ics[icc][bass.ds(rv, D), :]
                            .rearrange("(t p) f -> p t f", p=P),
                    )
                    for oc in range(4):
                        y_ps = d_ps.tile([P, 512], F32, name=f"yp{icc}_{oc}", tag="yp")
                        for mt in range(CT):
                            nc.tensor.matmul(
                                y_ps[:],
                                wo_sb[:, mt, oc * P:(oc + 1) * P],
                                ag_ch[:, mt, :],
                                start=(mt == 0), stop=(mt == CT - 1),
                            )
                        y_sb = ev2.tile([P, 512], F32, name=f"ysb{icc}_{oc}", tag="ysb")
                        nc.scalar.copy(y_sb[:], y_ps[:])
                        nc.scalar.dma_start(
                            out=yt_out[oc * P:(oc + 1) * P, icc * 512:(icc + 1) * 512],
                            in_=y_sb[:],
                        )

    nc.compile()
    _program_cache["nc"] = nc
    return nc


def _rope_tables():
    inv_freq = 1.0 / (10000.0 ** (np.arange(0, HD, 2, dtype=np.float32) / HD))
    pos = np.arange(S, dtype=np.float32)
    freqs = np.outer(pos, inv_freq).astype(np.float32)
    return np.cos(freqs).astype(np.float16), np.sin(freqs).astype(np.float16)


def _mask_tiles():
    m = np.zeros((4, P, 512), dtype=np.float16)
    jj = np.arange(P)[:, None]
    ii = np.arange(512)[None, :]
    for t in range(4):
        m[t] = np.where(t * P + jj > ii, 0.0, 1.0)
    return m


def make_in_maps(x, Wq, Wk, Wv, Wo):
    x = np.asarray(x, dtype=np.float32)
    cos, sin = _rope_tables()
    maskt = _mask_tiles()
    wqT = np.ascontiguousarray(np.asarray(Wq, dtype=np.float32).T.astype(np.float16))
    wkT = np.ascontiguousarray(np.asarray(Wk, dtype=np.float32).T.astype(np.float16))
    wvT = np.ascontiguousarray(np.asarray(Wv, dtype=np.float32).T.astype(np.float16))
    woT = np.ascontiguousarray(np.asarray(Wo, dtype=np.float32).T.astype(np.float16))
    xts = [np.ascontiguousarray(x[b].T.astype(np.float16)) for b in range(2)]
    in_maps = []
    for c in range(NCORES):
        b, g = c // 4, c % 4
        sl = slice(g * GW, (g + 1) * GW)
        in_maps.append({
            "roff": np.array([[b * D, 0]], dtype=np.uint32),
            "xt": xts[b],
            "wq": np.ascontiguousarray(wqT[:, sl]),
            "wk": np.ascontiguousarray(wkT[:, sl]),
            "wv": np.ascontiguousarray(wvT[:, sl]),
            "wo": np.ascontiguousarray(woT[:, sl]),
            "cos": cos,
            "sin": sin,
            "maskt": maskt,
        })
    return in_maps


def assemble_output(results):
    y = np.empty((2, S, D), dtype=np.float32)
    for c in range(NCORES):
        b, g = c // 4, c % 4
        y[b][:, g * GW:(g + 1) * GW] = results[c]["yt_out"].T
    return y


def kernel(x, Wq, Wk, Wv, Wo):
    nc = build_program()
    in_maps = make_in_maps(x, Wq, Wk, Wv, Wo)
    res = run_bass_kernel_spmd(nc, in_maps, core_ids=list(range(NCORES)))
    return assemble_output(res.results)


# revision 18
# speedup vs baseline: 1.0055x; 1.0055x over previous
"""Causal self-attention (QK-RMSNorm + RoPE) on 8 Trainium2 NeuronCores.

Problem: x[2,2048,2048], Wq/Wk/Wv/Wo [2048,2048], 16 heads, head_dim 128.

Sharding: core c handles batch b=c//4 and head group g=c%4 (4 heads,
model cols [512g:512g+512)).  QKV projections are computed from the
host-pre-transposed xT (contraction dim on partitions).  Attention uses
transposed scores (eT = exp(scale * kT_blk.T @ qT_chunk)), so the AV
matmul (lhsT=v, rhs=eT) directly yields the transposed attention output
yT[d, i] that o_proj consumes.  The softmax denominator comes from a
ones-lhsT matmul over eT, its reciprocal is broadcast across partitions
with a K=1 PE matmul.  Per-batch groups of 4 cores AllGather their
yT head shards, then each core computes a 512-column slice of the
output projection (y = attn @ Wo.T) in transposed layout.  The host
de-transposes and concatenates.  No AllReduce is needed.

Matmuls run in fp32r (full PE rate, ~1.5e-4 rel err).
"""

import math
from contextlib import ExitStack

import numpy as np

import concourse.bass as bass
import concourse.bacc as bacc
import concourse.tile as tile
from concourse import mybir
from concourse.bass_utils import run_bass_kernel_spmd
from concourse.masks import make_identity

P = 128
D = 2048
S = 2048
HD = 128              # head dim
NHL = 4               # heads per core
GW = NHL * HD         # 512, per-core width of head group
CT = D // P           # 16 contraction tiles
ICH = 4               # i-chunks of 512 positions
NCORES = 8
F32 = mybir.dt.float32
F16 = mybir.dt.float16
F32R = mybir.dt.float32r
SCALE = 1.0 / math.sqrt(HD)
EPS = 1.1920928955078125e-07
MASK_NEG = -30000.0

_program_cache = {}


def build_program():
    if "nc" in _program_cache:
        return _program_cache["nc"]

    nc = bacc.Bacc("TRN2", target_bir_lowering=False, debug=False, num_devices=NCORES)

    xt_in = nc.dram_tensor("xt", [D, S], F16, kind="ExternalInput")
    wq_in = nc.dram_tensor("wq", [D, GW], F16, kind="ExternalInput")
    wk_in = nc.dram_tensor("wk", [D, GW], F16, kind="ExternalInput")
    wv_in = nc.dram_tensor("wv", [D, GW], F16, kind="ExternalInput")
    wo_in = nc.dram_tensor("wo", [D, GW], F16, kind="ExternalInput")
    cos_in = nc.dram_tensor("cos", [S, HD // 2], F16, kind="ExternalInput")
    sin_in = nc.dram_tensor("sin", [S, HD // 2], F16, kind="ExternalInput")
    mask_in = nc.dram_tensor("maskt", [4, P, 512], F16, kind="ExternalInput")
    roff_in = nc.dram_tensor("roff", [1, 2], mybir.dt.uint32, kind="ExternalInput")
    yt_out = nc.dram_tensor("yt_out", [GW, S], F32, kind="ExternalOutput")

    with tile.TileContext(nc) as tc:
        with ExitStack() as ctx:
            const = ctx.enter_context(tc.tile_pool(name="const", bufs=1))
            dram = ctx.enter_context(tc.tile_pool(name="dram", bufs=1, space="DRAM"))

            ident = const.tile([P, P], F16, name="ident")
            make_identity(nc, ident)
            eps_t = const.tile([P, 1], F32, name="eps_t")
            nc.vector.memset(eps_t[:], EPS)
            zero_t = const.tile([P, 1], F32, name="zero_t")
            nc.vector.memset(zero_t[:], 0.0)
            neg1_t = const.tile([P, 1], F32, name="neg1_t")
            nc.vector.memset(neg1_t[:], -1.0)
            ones_f = const.tile([P, P], F32, name="ones_f")
            nc.vector.memset(ones_f[:], 1.0)
            ones2 = const.tile([P, 2], F16, name="ones2")
            nc.scalar.copy(ones2[:], ones_f[:, 0:2])
            ones_row = const.tile([1, P], F32R, name="ones_row")
            nc.scalar.copy(ones_row[:], ones_f[0:1, :])

            cos_sb = const.tile([P, CT, HD // 2], F16, name="cos_sb")
            nc.sync.dma_start(out=cos_sb[:], in_=cos_in.ap().rearrange("(a p) f -> p a f", p=P))
            sin_sb = const.tile([P, CT, HD // 2], F16, name="sin_sb")
            nc.sync.dma_start(out=sin_sb[:], in_=sin_in.ap().rearrange("(a p) f -> p a f", p=P))

            qt_d = dram.tile([GW, S], F16, name="qt_d")
            kt_d = dram.tile([GW, S], F16, name="kt_d")
            yt_ics = [dram.tile([GW, 512], F16, name=f"yt_ic{i}") for i in range(ICH)]
            ag_ics = [
                dram.tile([NCORES * GW, 512], F16, name=f"ag_ic{i}", addr_space="Shared")
                for i in range(ICH)
            ]

            # wo pool created up-front so phase A can prefetch into it
            wopool = ctx.enter_context(tc.tile_pool(name="wopool", bufs=1))
            wo_sb = wopool.tile([P, CT, GW], F16, name="wo_sb")

            # ---------------- Phase A: Q and K ----------------
            with ExitStack() as pha:
                wpool = pha.enter_context(tc.tile_pool(name="wpool", bufs=1))
                xt_pool = pha.enter_context(tc.tile_pool(name="xt_pool", bufs=2))
                proj_ps = pha.enter_context(tc.tile_pool(name="proj_ps", bufs=2, space="PSUM"))
                tp_ps = pha.enter_context(tc.tile_pool(name="tp_ps", bufs=2, space="PSUM"))
                rope = pha.enter_context(tc.tile_pool(name="rope", bufs=2))
                stat = pha.enter_context(tc.tile_pool(name="stat", bufs=2))
                evq = pha.enter_context(tc.tile_pool(name="evq", bufs=3))

                wq_sb = wpool.tile([P, CT, GW], F16, name="wq_sb")
                wk_sb = wpool.tile([P, CT, GW], F16, name="wk_sb")
                # prefetch O weights late in phase A (pool lives in outer scope)
                for ica in range(8):
                    xt_ch = xt_pool.tile([P, CT, 256], F16, name=f"xt_ch{ica}", tag="xt")
                    for ct in range(CT):
                        if ica == 0:
                            # interleave weight-tile and first-chunk loads so the
                            # first accumulation groups unblock ct-by-ct
                            nc.sync.dma_start(out=wq_sb[:, ct, :], in_=wq_in[ct * P:(ct + 1) * P, :])
                        nc.sync.dma_start(
                            out=xt_ch[:, ct, :],
                            in_=xt_in[ct * P:(ct + 1) * P, ica * 256:(ica + 1) * 256],
                        )
                        if ica == 0:
                            nc.sync.dma_start(out=wk_sb[:, ct, :], in_=wk_in[ct * P:(ct + 1) * P, :])
                        if ica == 1:
                            nc.sync.dma_start(out=wo_sb[:, ct, :], in_=wo_in[ct * P:(ct + 1) * P, :])
                    for ib in range(2):
                        ibg = ica * 2 + ib         # global i-block
                        i0 = ibg * P
                        for wsb, dst in ((wq_sb, qt_d), (wk_sb, kt_d)):
                            ps = proj_ps.tile([P, GW], F32, name=f"ps{ibg}", tag="proj")
                            for ct in range(CT):
                                nc.tensor.matmul(
                                    ps[:],
                                    xt_ch[:, ct, ib * P:(ib + 1) * P],
                                    wsb[:, ct, :],
                                    start=(ct == 0),
                                    stop=(ct == CT - 1),
                                )
                            # rms norm stats + rope, all row-wise
                            qs = rope.tile([P, GW], F16, name=f"qs{ibg}", tag="qs")
                            nc.scalar.copy(qs[:], ps[:])
                            sq = rope.tile([P, GW], F16, name=f"sq{ibg}", tag="sq")
                            nc.vector.tensor_mul(sq[:], qs[:], qs[:])
                            rstd = stat.tile([P, NHL], F32, name=f"rstd{ibg}", tag="rstd")
                            for h in range(NHL):
                                nc.vector.reduce_sum(
                                    rstd[:, h:h + 1],
                                    sq[:, h * HD:(h + 1) * HD],
                                    axis=mybir.AxisListType.X,
                                )
                            nc.scalar.activation(
                                rstd[:], rstd[:],
                                mybir.ActivationFunctionType.Sqrt,
                                bias=eps_t[:], scale=1.0 / HD,
                            )
                            nc.vector.reciprocal(rstd[:], rstd[:])

                            q3 = qs[:].rearrange("p (h d) -> p h d", h=NHL)
                            qr = rope.tile([P, GW], F16, name=f"qr{ibg}", tag="qr")
                            qr3 = qr[:].rearrange("p (h d) -> p h d", h=NHL)
                            tmp = rope.tile([P, NHL, HD // 2], F16, name=f"tmp{ibg}", tag="tmp")
                            cosB = cos_sb[:, ibg:ibg + 1, :].broadcast_to((P, NHL, HD // 2))
                            sinB = sin_sb[:, ibg:ibg + 1, :].broadcast_to((P, NHL, HD // 2))
                            h1 = q3[:, :, 0:HD // 2]
                            h2 = q3[:, :, HD // 2:HD]
                            # r1 = q1*cos + q2*sin ; r2 = q2*cos - q1*sin
                            nc.vector.tensor_mul(qr3[:, :, 0:HD // 2], h1, cosB)
                            nc.vector.tensor_mul(tmp[:], h2, sinB)
                            nc.vector.tensor_add(qr3[:, :, 0:HD // 2], qr3[:, :, 0:HD // 2], tmp[:])
                            nc.vector.tensor_mul(qr3[:, :, HD // 2:HD], h2, cosB)
                            nc.vector.tensor_mul(tmp[:], h1, sinB)
                            nc.vector.tensor_sub(
                                qr3[:, :, HD // 2:HD], qr3[:, :, HD // 2:HD], tmp[:]
                            )
                            for h in range(NHL):
                                nc.vector.tensor_scalar_mul(
                                    qr[:, h * HD:(h + 1) * HD],
                                    qr[:, h * HD:(h + 1) * HD],
                                    rstd[:, h:h + 1],
                                )
                            # transpose per head, evict to DRAM
                            for h in range(NHL):
                                tp = tp_ps.tile([P, P], F16, name=f"tp{ibg}_{h}", tag="tp")
                                nc.tensor.transpose(tp[:], qr[:, h * HD:(h + 1) * HD], ident[:])
                                qtt = evq.tile([P, P], F16, name=f"qtt{ibg}_{h}", tag="qtt")
                                nc.scalar.copy(qtt[:], tp[:])
                                nc.scalar.dma_start(
                                    out=dst[h * P:(h + 1) * P, i0:i0 + P], in_=qtt[:]
                                )

            # wv/wo/v pools: wo persists to phase D; wv+v live A..B
            # (their DMAs are issued inside phase A for prefetch)

            # ---------------- Phase A2: V ----------------
            with ExitStack() as phab:
                # v + mask live A2..B; freed before phase D
                vpool = phab.enter_context(tc.tile_pool(name="vpool", bufs=1))
                v_sb = vpool.tile([P, CT, GW], F16, name="v_sb")
                maskp = phab.enter_context(tc.tile_pool(name="maskp", bufs=1))
                mask_sb = maskp.tile([P, 4, 512], F16, name="mask_sb")
                nc.sync.dma_start(out=mask_sb[:], in_=mask_in.ap().rearrange("t p f -> p t f"))

                with ExitStack() as phv:
                    wvpool = phv.enter_context(tc.tile_pool(name="wvpool", bufs=1))
                    xt2_pool = phv.enter_context(tc.tile_pool(name="xt2_pool", bufs=2))
                    v_ps = phv.enter_context(tc.tile_pool(name="v_ps", bufs=2, space="PSUM"))
                    wv_sb = wvpool.tile([P, CT, GW], F16, name="wv_sb")
                    for ct in range(CT):
                        nc.sync.dma_start(out=wv_sb[:, ct, :], in_=wv_in[ct * P:(ct + 1) * P, :])

                    for ic2 in range(8):
                        xt_ch = xt2_pool.tile([P, CT, 256], F16, name=f"x2_{ic2}", tag="xt2")
                        for ct in range(CT):
                            nc.sync.dma_start(
                                out=xt_ch[:, ct, :],
                                in_=xt_in[ct * P:(ct + 1) * P, ic2 * 256:(ic2 + 1) * 256],
                            )
                        for ib in range(2):
                            jb = ic2 * 2 + ib
                            ps = v_ps.tile([P, GW], F32, name=f"vps{jb}", tag="vps")
                            for ct in range(CT):
                                nc.tensor.matmul(
                                    ps[:],
                                    xt_ch[:, ct, ib * P:(ib + 1) * P],
                                    wv_sb[:, ct, :],
                                    start=(ct == 0),
                                    stop=(ct == CT - 1),
                                )
                            nc.scalar.copy(v_sb[:, jb, :], ps[:])

                # ---------------- Phase B: attention, i-chunk outer ----------------
                with ExitStack() as phb:
                    kt_pool = phb.enter_context(tc.tile_pool(name="kt_pool", bufs=2))
                    qt_pool = phb.enter_context(tc.tile_pool(name="qt_pool", bufs=2))
                    et_pool = phb.enter_context(tc.tile_pool(name="et_pool", bufs=4))
                    s_ps = phb.enter_context(tc.tile_pool(name="s_ps", bufs=3, space="PSUM"))
                    acc_ps = phb.enter_context(tc.tile_pool(name="acc_ps", bufs=2, space="PSUM"))
                    bc_psp = phb.enter_context(tc.tile_pool(name="bc_psp", bufs=1, space="PSUM"))
                    bsmall = phb.enter_context(tc.tile_pool(name="bsmall", bufs=2))

                    for ic in reversed(range(ICH)):
                        njb = 4 * ic + 4
                        kt_t = kt_pool.tile([P, NHL, 2048], F16, name=f"kt{ic}", tag="kt")
                        qt_t = qt_pool.tile([P, NHL, 512], F16, name=f"qt{ic}", tag="qt")
                        for h in range(NHL):
                            nc.sync.dma_start(
                                out=kt_t[:, h, 0:njb * P],
                                in_=kt_d[h * P:(h + 1) * P, 0:njb * P],
                            )
                            nc.sync.dma_start(
                                out=qt_t[:, h, :],
                                in_=qt_d[h * P:(h + 1) * P, ic * 512:(ic + 1) * 512],
                            )
                        for h in range(NHL):
                            yt_ps = acc_ps.tile([P, 512], F32, name=f"yt{h}_{ic}", tag="yt")
                            den_ps = acc_ps.tile([2, 512], F32, name=f"den{h}_{ic}", tag="den")
                            for jb in range(njb):
                                sp = s_ps.tile([P, 512], F32, name=f"s{h}_{ic}_{jb}", tag="s")
                                nc.tensor.matmul(
                                    sp[:],
                                    kt_t[:, h, jb * P:(jb + 1) * P],
                                    qt_t[:, h, :],
                                    start=True, stop=True,
                                )
                                et = et_pool.tile([P, 512], F16, name=f"et{h}_{ic}_{jb}", tag="et")
                                nc.scalar.activation(
                                    et[:], sp[:],
                                    mybir.ActivationFunctionType.Exp,
                                    bias=neg1_t[:], scale=SCALE,
                                )
                                t = jb - 4 * ic
                                if t >= 0:
                                    nc.vector.tensor_mul(et[:], et[:], mask_sb[:, t, :])
                                nc.tensor.matmul(
                                    yt_ps[:],
                                    v_sb[:, jb, h * HD:(h + 1) * HD],
                                    et[:],
                                    start=(jb == 0), stop=(jb == njb - 1),
                                )
                                nc.tensor.matmul(
                                    den_ps[:],
                                    ones2[:],
                                    et[:],
                                    start=(jb == 0), stop=(jb == njb - 1),
                                )
                            rden = bsmall.tile([1, 512], F32, name=f"rd{h}_{ic}", tag="rden")
                            nc.vector.reciprocal(rden[:], den_ps[0:1, :])
                            rden_r = bsmall.tile([1, 512], F32R, name=f"rdr{h}_{ic}", tag="rdenr")
                            nc.scalar.copy(rden_r[:], rden[:])
                            bc_ps = bc_psp.tile([P, 512], F32, name=f"bc{h}_{ic}", tag="bc")
                            nc.tensor.matmul(bc_ps[:], ones_row[:], rden_r[:], start=True, stop=True)
                            bc_sb = bsmall.tile([P, 512], F32, name=f"bcs{h}_{ic}", tag="bcs")
                            nc.vector.tensor_copy(bc_sb[:], bc_ps[:])
                            yt_sb = bsmall.tile([P, 512], F16, name=f"yts{h}_{ic}", tag="yts")
                            nc.vector.tensor_mul(yt_sb[:], yt_ps[:], bc_sb[:])
                            nc.gpsimd.dma_start(
                                out=yt_ics[ic][h * P:(h + 1) * P, :],
                                in_=yt_sb[:],
                            )
                        # per-chunk AllGather fires as soon as chunk ic is written
                        nc.gpsimd.collective_compute(
                            "AllGather",
                            mybir.AluOpType.bypass,
                            replica_groups=[list(range(NCORES))],
                            ins=[yt_ics[ic][:].opt()],
                            outs=[ag_ics[ic][:].opt()],
                        )

            # ---------------- Phase D: o_proj (column shard), pipelined per chunk ----------------
            with ExitStack() as phd:
                ag_pool = phd.enter_context(tc.tile_pool(name="ag_pool", bufs=2))
                d_ps = phd.enter_context(tc.tile_pool(name="d_ps", bufs=2, space="PSUM"))
                ev2 = phd.enter_context(tc.tile_pool(name="ev2", bufs=3))
                roffp = phd.enter_context(tc.tile_pool(name="roffp", bufs=1))

                roff_sb = roffp.tile([1, 2], mybir.dt.uint32, name="roff_sb")
                nc.sync.dma_start(out=roff_sb[:], in_=roff_in[:, :])
                roff_reg = nc.alloc_registers()
                nc.regs_load(roff_reg, roff_sb[0:1, 0:1])
                rv = nc.snap(roff_reg, donate=True)

                for icc in reversed(range(ICH)):
                    ag_ch = ag_pool.tile([P, CT, 512], F16, name=f"ag{icc}", tag="ag")
                    nc.sync.dma_start(
                        out=ag_ch[:],
                        in_=ag_ics[icc][bass.ds(rv, D), :]
                            .rearrange("(t p) f -> p t f", p=P),
                    )
                    for oc in range(4):
                        y_ps = d_ps.tile([P, 512], F32, name=f"yp{icc}_{oc}", tag="yp")
                        for mt in range(CT):
                            nc.tensor.matmul(
                                y_ps[:],
                                wo_sb[:, mt, oc * P:(oc + 1) * P],
                                ag_ch[:, mt, :],
                                start=(mt == 0), stop=(mt == CT - 1),
                            )
                        y_sb = ev2.tile([P, 512], F32, name=f"ysb{icc}_{oc}", tag="ysb")
                        nc.scalar.copy(y_sb[:], y_ps[:])
                        nc.scalar.dma_start(
                            out=yt_out[oc * P:(oc + 1) * P, icc * 512:(icc + 1) * 512],
                            in_=y_sb[:],
                        )

    nc.compile()
    _program_cache["nc"] = nc
    return nc


def _rope_tables():
    inv_freq = 1.0 / (10000.0 ** (np.arange(0, HD, 2, dtype=np.float32) / HD))
    pos = np.arange(S, dtype=np.float32)
    freqs = np.outer(pos, inv_freq).astype(np.float32)
    return np.cos(freqs).astype(np.float16), np.sin(freqs).astype(np.float16)


def _mask_tiles():
    m = np.zeros((4, P, 512), dtype=np.float16)
    jj = np.arange(P)[:, None]
    ii = np.arange(512)[None, :]
    for t in range(4):
        m[t] = np.where(t * P + jj > ii, 0.0, 1.0)
    return m


def make_in_maps(x, Wq, Wk, Wv, Wo):
    x = np.asarray(x, dtype=np.float32)
    cos, sin = _rope_tables()
    maskt = _mask_tiles()
    wqT = np.ascontiguousarray(np.asarray(Wq, dtype=np.float32).T.astype(np.float16))
    wkT = np.ascontiguousarray(np.asarray(Wk, dtype=np.float32).T.astype(np.float16))
    wvT = np.ascontiguousarray(np.asarray(Wv, dtype=np.float32).T.astype(np.float16))
    woT = np.ascontiguousarray(np.asarray(Wo, dtype=np.float32).T.astype(np.float16))
    xts = [np.ascontiguousarray(x[b].T.astype(np.float16)) for b in range(2)]
    in_maps = []
    for c in range(NCORES):
        b, g = c // 4, c % 4
        sl = slice(g * GW, (g + 1) * GW)
        in_maps.append({
            "roff": np.array([[b * D, 0]], dtype=np.uint32),
            "xt": xts[b],
            "wq": np.ascontiguousarray(wqT[:, sl]),
            "wk": np.ascontiguousarray(wkT[:, sl]),
            "wv": np.ascontiguousarray(wvT[:, sl]),
            "wo": np.ascontiguousarray(woT[:, sl]),
            "cos": cos,
            "sin": sin,
            "maskt": maskt,
        })
    return in_maps


def assemble_output(results):
    y = np.empty((2, S, D), dtype=np.float32)
    for c in range(NCORES):
        b, g = c // 4, c % 4
        y[b][:, g * GW:(g + 1) * GW] = results[c]["yt_out"].T
    return y


def kernel(x, Wq, Wk, Wv, Wo):
    nc = build_program()
    in_maps = make_in_maps(x, Wq, Wk, Wv, Wo)
    res = run_bass_kernel_spmd(nc, in_maps, core_ids=list(range(NCORES)))
    return assemble_output(res.results)
